# revision 11
# baseline (speedup 1.0000x reference)
"""Trainium2 Bass kernel for nn_MentionScore.

Strategy: sort spans by start, shard 2048 consecutive sorted spans per core.
Each core only touches a ~1.2k-token window of states/embeds. The ragged
gather/softmax/weighted-sum becomes dense matmuls against one-hot / banded
matrices built on-device with iota-compare vector ops. Layer-1 of the span
MLP is algebraically folded:
  h1 = relu(OH_s.T@P1 + OH_e.T@P2 + Wg.T@P3 + onehot(len).T@WB)
with P1=states@W1a, P2=states@W1b, P3=embeds@W1c precomputed per token
(kept in SBUF, group windows 128-aligned) and WB = width_table@W1d + b1.

The attention-logit MLP runs in fp8 (e4m3, weights pre-scaled by 32) with
DoubleRow matmuls; exp(logits) is produced directly by the scalar engine so
the span stage builds the normalized band matrix with two range-compares
against a broadcast exp row (no per-group softmax). Band matrices are
transposed by the DMA XBAR instead of the PE array.
"""

import sys
import types

import numpy as np
import ml_dtypes

import concourse.bass as bass
import concourse.mybir as mybir
from concourse.ap import AP
from concourse.tile import TileContext
from concourse.vector_clock import ScopedClock

BF = mybir.dt.bfloat16
F32 = mybir.dt.float32
F8 = mybir.dt.float8e4
AT = mybir.AluOpType
AF = mybir.ActivationFunctionType
AX = mybir.AxisListType
PM = mybir.MatmulPerfMode
bf16 = ml_dtypes.bfloat16
f8e4 = ml_dtypes.float8_e4m3

N_CORES = 8
T, NSPAN, D, HID, LMAX, WD = 8192, 16384, 1024, 1024, 10, 20
C = NSPAN // N_CORES          # spans per core
G = C // 128                  # 128-span groups per core
FS = 32.0                     # fp8 weight prescale


class PatchedTileContext(TileContext):
    """Workaround: walrus rejects the tail Drain when it carries >1 sem wait
    ("Too many sync wait commands"). Put each wait on its own NoOp instead."""

    def _drain_and_barrier(self, tick_clock, wait_clock):
        nc = self.nc
        drain_inst = nc.sync.drain()
        wait_clock.add_sem_waits(
            drain_inst.ins, ScopedClock({None: tick_clock.global_clock})
        )
        si = drain_inst.ins.sync_info
        if si is not None and si.on_wait is not None and len(si.on_wait) > 1:
            waits = list(si.on_wait)
            drain_inst.ins.sync_info = mybir.SyncInfo(
                on_wait=[waits[0]], on_update=list(si.on_update or [])
            )
            for w in waits[1:]:
                nop = nc.sync.nop()
                nop.ins.sync_info = mybir.SyncInfo(on_wait=[w], on_update=[])

        nc.all_engine_barrier()
        assert self.sems is not None
        popped = nc._tile_sem_poison_stack.pop()
        assert popped is self._sem_poison
        nc.clear_and_free_semaphores(list(self.sems.allocated().values()))
        nc.all_engine_barrier()


def _ceil128(x):
    return int(-(-int(x) // 128) * 128)


def _plan(span_starts, span_lengths):
    """Host-side sharding plan. Returns per-core data + static layout consts."""
    order = np.argsort(span_starts, kind="stable").astype(np.int64)
    ss = span_starts[order].reshape(N_CORES, C).astype(np.int64)
    sl = span_lengths[order].reshape(N_CORES, C).astype(np.int64)
    core_base = ss[:, 0].copy()
    sloc = ss - core_base[:, None]
    eloc = sloc + sl

    T_cap = _ceil128(int(eloc.max()) + 1)
    # 128-aligned shared-across-cores group window bases + per-group k-tiles
    mn = sloc.reshape(N_CORES, G, 128).min(axis=2).min(axis=0)   # [G]
    mx = eloc.reshape(N_CORES, G, 128).max(axis=2).max(axis=0)   # [G]
    bases = (mn // 128) * 128
    kcs = -(-(mx - bases + 1) // 128)
    d = sloc - np.repeat(bases, 128)[None, :]
    assert d.min() >= 0
    assert ((d + sl) <= np.repeat(kcs, 128)[None, :] * 128 - 1).all()

    return {
        "order": order,
        "core_base": core_base,
        "d": d.astype(np.float64),
        "dl": (d + sl).astype(np.float64),
        "ln": sl.astype(np.float64),
        "T_cap": T_cap,
        "bases": [int(b) for b in bases],
        "kcs": [int(k) for k in kcs],
    }


NGROUPS = G
SPLIT_WAITS = True


def _build(T_cap, bases, kcs, b3val, ab3val):
    """Build the single SPMD Bass program (static; shared by all 8 cores)."""
    TC = T_cap // 128
    NCH = TC + 2                      # P chunks incl zero pad
    KC = max(kcs)
    K_WIN = KC * 128
    NB = -(-T_cap // 256)             # 256-token blocks
    T_pad2 = (NCH + 1) * 128
    nc = bass.Bass()

    def par(name, shape, dt):
        return nc.declare_dram_parameter(name, list(shape), dt, isOutput=False)

    statesTb_p = par("statesTb", [128, NB, 8, 256], BF)
    embedsTb_p = par("embedsTb", [128, NB, 8, 256], BF)
    dmat_p = par("dmat", [128, G], F32)
    dlmat_p = par("dlmat", [128, G], F32)
    dflat_p = par("dflat", [1, C], F32)
    deflat_p = par("deflat", [1, C], F32)
    lenflat_p = par("lenflat", [1, C], F32)
    aw1_p = par("aw1", [128, 8 * HID], F8)
    aw2_p = par("aw2", [128, 8 * HID], F8)
    aw3_p = par("aw3", [128, 8, 1], F8)
    ab1_p = par("ab1m", [128, 8], F32)
    ab2_p = par("ab2m", [128, 8], F32)
    w1a_p = par("w1a", [128, 8 * HID], BF)
    w1b_p = par("w1b", [128, 8 * HID], BF)
    w1c_p = par("w1c", [128, 8 * HID], BF)
    w1d_p = par("w1d", [WD, HID], BF)
    wtT_p = par("wtT", [WD, LMAX], BF)
    b1r_p = par("b1r", [1, HID], BF)
    w2_p = par("w2", [128, 8 * HID], BF)
    b2_p = par("b2m", [128, 8], F32)
    w3_p = par("w3m", [128, 8], BF)
    iotaK_p = par("iotaK", [1, K_WIN], F32)
    iotaC_p = par("iotaC", [128, KC], F32)
    scores_p = nc.declare_dram_parameter("scores", [1, C], F32, isOutput=True)

    with PatchedTileContext(nc) as tc:
        with (
            tc.tile_pool(name="pp", bufs=1) as pp,
            tc.tile_pool(name="ps", bufs=2, space="PSUM") as ps,
            tc.tile_pool(name="dp", bufs=1, space="DRAM") as dp,
        ):
            dma = nc.sync.dma_start

            expa_d = dp.tile([1, T_pad2], F32, name="expa_d", tag="expa_d")

            # ---------- persistent tiles ----------
            P1 = pp.tile([128, NCH, HID], BF, name="P1", tag="P1")
            P2 = pp.tile([128, NCH, HID], BF, name="P2", tag="P2")
            P3 = pp.tile([128, NCH, HID], BF, name="P3", tag="P3")
            Pmats = (P1, P2, P3)
            w2_t = pp.tile([128, 8, HID], BF, name="w2", tag="w2")
            w3_t = pp.tile([128, 8], BF, name="w3", tag="w3")
            b2_t = pp.tile([128, 8], F32, name="b2", tag="b2")
            WBfull = pp.tile([128, HID], BF, name="WBfull", tag="WBfull")
            iotaC_t = pp.tile([128, KC], F32, name="iotaC", tag="iotaC")
            iotaKr = pp.tile([128, K_WIN], F32, name="iotaKr", tag="iotaKr")
            dmat_t = pp.tile([128, G], F32, name="dmat", tag="dmat")
            dlmat_t = pp.tile([128, G], F32, name="dlmat", tag="dlmat")

            with tc.tile_pool(name="tk", bufs=1) as tk:
                # first DMA wave: exactly what block 0 of the token stage
                # needs, split across queues (round-robin -> parallel)
                aw1_t = tk.tile([128, 8, HID], F8, name="aw1", tag="aw1")
                for q in range(4):
                    dma(out=aw1_t[:, 2 * q:2 * q + 2, :],
                        in_=aw1_p[:, 2 * q * HID:(2 * q + 2) * HID])

                sTb = [None] * NB
                eTb = [None] * NB

                def load_block(b):
                    n0 = b * 256
                    nw = min(256, T_cap - n0)
                    sTb[b] = tk.tile([128, 8, 256], BF, name=f"sTb",
                                     tag="sTb", bufs=3)
                    dma(out=sTb[b][:, 0:4, :nw], in_=statesTb_p[:, b, 0:4, :nw])
                    dma(out=sTb[b][:, 4:8, :nw], in_=statesTb_p[:, b, 4:8, :nw])
                    eTb[b] = tk.tile([128, 8, 256], BF, name=f"eTb",
                                     tag="eTb", bufs=3)
                    dma(out=eTb[b][:, 0:4, :nw], in_=embedsTb_p[:, b, 0:4, :nw])
                    dma(out=eTb[b][:, 4:8, :nw], in_=embedsTb_p[:, b, 4:8, :nw])

                load_block(0)
                ab1_t = tk.tile([128, 8], F32, name="ab1", tag="ab1")
                dma(out=ab1_t[:], in_=ab1_p[:])
                ab2_t = tk.tile([128, 8], F32, name="ab2", tag="ab2")
                dma(out=ab2_t[:], in_=ab2_p[:])
                aw3_t = tk.tile([128, 8, 1], F8, name="aw3", tag="aw3")
                dma(out=aw3_t[:], in_=aw3_p[:])
                dma(out=iotaC_t[:], in_=iotaC_p[:])

                # second wave: weights for the rest of the pipeline
                aw2_t = tk.tile([128, 8, HID], F8, name="aw2", tag="aw2")
                for q in range(2):
                    dma(out=aw2_t[:, 4 * q:4 * q + 4, :],
                        in_=aw2_p[:, 4 * q * HID:(4 * q + 4) * HID])
                w1_t = []
                for i, p_ in enumerate((w1a_p, w1b_p, w1c_p)):
                    t = tk.tile([128, 8, HID], BF, name=f"w1_{i}", tag=f"w1_{i}")
                    for q in range(2):
                        dma(out=t[:, 4 * q:4 * q + 4, :],
                            in_=p_[:, 4 * q * HID:(4 * q + 4) * HID])
                    w1_t.append(t)
                load_block(1)
                for q in range(2):
                    dma(out=w2_t[:, 4 * q:4 * q + 4, :],
                        in_=w2_p[:, 4 * q * HID:(4 * q + 4) * HID])
                dma(out=w3_t[:], in_=w3_p[:])
                dma(out=b2_t[:], in_=b2_p[:])
                dma(out=iotaKr[:], in_=iotaK_p[:].partition_broadcast(128))
                dma(out=dmat_t[:], in_=dmat_p[:])
                dma(out=dlmat_t[:], in_=dlmat_p[:])
                wtT_t = tk.tile([WD, 16], BF, name="wtT", tag="wtT")
                nc.vector.memset(wtT_t[:], 0.0)
                dma(out=wtT_t[:, :LMAX], in_=wtT_p[:])
                w1d_t = tk.tile([WD, HID], BF, name="w1d", tag="w1d")
                dma(out=w1d_t[:], in_=w1d_p[:])
                b1r_t = tk.tile([1, HID], BF, name="b1r", tag="b1r")
                dma(out=b1r_t[:], in_=b1r_p[:])
                ones16_t = tk.tile([1, 16], BF, name="ones16", tag="ones16")
                nc.vector.memset(ones16_t[:], 1.0)

                # zero-fill upper P chunks + expa pad once
                nc.vector.memset(P1[:, TC:, :], 0.0)
                nc.vector.memset(P2[:, TC:, :], 0.0)
                nc.vector.memset(P3[:, TC:, :], 0.0)
                zpad = tk.tile([1, T_pad2 - T_cap], F32, name="zpad", tag="zpad")
                nc.vector.memset(zpad[:], 0.0)
                dma(out=expa_d[0:1, T_cap:], in_=zpad[:])

                # WBfull = [width_table@W1d + b1 ; 0] as [128(len pad), HID]
                nc.vector.memset(WBfull[:], 0.0)
                for h0 in (0, 512):
                    wbp = ps.tile([16, 512], F32, name="wbp", tag="wbp", bufs=1)
                    nc.tensor.matmul(wbp[:], wtT_t[:], w1d_t[:, h0:h0 + 512],
                                     start=True, stop=False)
                    nc.tensor.matmul(wbp[:], ones16_t[:], b1r_t[:, h0:h0 + 512],
                                     start=False, stop=True)
                    nc.scalar.copy(WBfull[0:16, h0:h0 + 512], wbp[:])

                # ---------- token pipeline ----------
                for b in range(NB):
                    n0 = b * 256
                    nw = min(256, T_cap - n0)
                    if b + 2 < NB:
                        load_block(b + 2)
                    sT8 = tk.tile([128, 8, 256], F8, name="sT8", tag="sT8",
                                  bufs=2)
                    nc.vector.tensor_copy(out=sT8[:, :, :nw],
                                          in_=sTb[b][:, :, :nw])
                    # attn l1 (fp8 DoubleRow)
                    h1a = tk.tile([128, 8, 256], F8, name="h1a", tag="h1a",
                                  bufs=2)
                    for hc in range(8):
                        pt = ps.tile([128, 512], F32, name="psA", tag="psA",
                                     bufs=2)
                        for jp in range(4):
                            nc.tensor.matmul(
                                pt[:, :nw],
                                aw1_t[:, 2 * jp:2 * jp + 2,
                                      hc * 128:(hc + 1) * 128],
                                sT8[:, 2 * jp:2 * jp + 2, :nw],
                                start=(jp == 0), stop=(jp == 3),
                                perf_mode=PM.DoubleRow)
                        nc.scalar.activation(h1a[:, hc, :nw], pt[:, :nw],
                                             AF.Relu, bias=ab1_t[:, hc:hc + 1])
                    # attn l2
                    h2a = tk.tile([128, 8, 256], F8, name="h2a", tag="h2a",
                                  bufs=2)
                    for hc in range(8):
                        pt = ps.tile([128, 512], F32, name="psA", tag="psA",
                                     bufs=2)
                        for jp in range(4):
                            nc.tensor.matmul(
                                pt[:, :nw],
                                aw2_t[:, 2 * jp:2 * jp + 2,
                                      hc * 128:(hc + 1) * 128],
                                h1a[:, 2 * jp:2 * jp + 2, :nw],
                                start=(jp == 0), stop=(jp == 3),
                                perf_mode=PM.DoubleRow)
                        nc.scalar.activation(h2a[:, hc, :nw], pt[:, :nw],
                                             AF.Relu, bias=ab2_t[:, hc:hc + 1],
                                             scale=1.0 / FS)
                    # attn l3 -> exp(logits)
                    pt = ps.tile([1, 512], F32, name="psL", tag="psL", bufs=1)
                    for k in range(8):
                        nc.tensor.matmul(
                            pt[:, :nw],
                            aw3_t[:, k, :],
                            h2a[:, k, :nw],
                            start=(k == 0), stop=(k == 7))
                    expb = tk.tile([1, 256], F32, name="expb", tag="expb",
                                   bufs=2)
                    nc.scalar.activation(expb[:, :nw], pt[:, :nw], AF.Exp,
                                         bias=float(ab3val),
                                         scale=1.0 / (FS * FS))
                    dma(out=expa_d[0:1, n0:n0 + nw], in_=expb[:, :nw])

                    # projections P1/P2/P3 (bf16)
                    for pi in range(3):
                        src = sTb[b] if pi < 2 else eTb[b]
                        for j in range(nw // 128):
                            ch = (n0 + j * 128) // 128
                            for h0 in (0, 512):
                                pt = ps.tile([128, 512], F32, name="psA",
                                             tag="psA", bufs=2)
                                for k in range(8):
                                    nc.tensor.matmul(
                                        pt[:],
                                        src[:, k, j * 128:(j + 1) * 128],
                                        w1_t[pi][:, k, h0:h0 + 512],
                                        start=(k == 0), stop=(k == 7))
                                nc.vector.tensor_copy(
                                    out=Pmats[pi][:, ch, h0:h0 + 512],
                                    in_=pt[:])

            # ---------- span stage ----------
            with tc.tile_pool(name="sp", bufs=1) as sp:
                h1big = h2big = None
                for g in range(NGROUPS):
                    KCg = kcs[g]
                    c0 = bases[g] // 128
                    W = KCg * 128
                    gcol = (g % 4) * 128
                    if g % 4 == 0:
                        h1big = sp.tile([128, 8, 512], BF, name="h1big",
                                        tag="h1big", bufs=2)

                    d_rep = sp.tile([128, 128], F32, name="d_rep",
                                    tag="d_rep", bufs=3)
                    dma(out=d_rep[:],
                        in_=dflat_p[:, g * 128:(g + 1) * 128]
                        .partition_broadcast(128))
                    de_rep = sp.tile([128, 128], F32, name="de_rep",
                                     tag="de_rep", bufs=3)
                    dma(out=de_rep[:],
                        in_=deflat_p[:, g * 128:(g + 1) * 128]
                        .partition_broadcast(128))
                    len_rep = sp.tile([128, 128], F32, name="len_rep",
                                      tag="len_rep", bufs=3)
                    dma(out=len_rep[:],
                        in_=lenflat_p[:, g * 128:(g + 1) * 128]
                        .partition_broadcast(128))
                    e_rep = sp.tile([128, K_WIN], F32, name="e_rep",
                                    tag="e_rep", bufs=3)
                    dma(out=e_rep[:, :W],
                        in_=expa_d[0:1, bases[g]:bases[g] + W]
                        .partition_broadcast(128))

                    # one-hots [tau, n]
                    ohT = sp.tile([128, K_WIN], BF, name="ohT", tag="ohT",
                                  bufs=3)
                    oheT = sp.tile([128, K_WIN], BF, name="oheT", tag="oheT",
                                   bufs=3)
                    for kk in range(KCg):
                        nc.vector.tensor_scalar(
                            out=ohT[:, kk * 128:(kk + 1) * 128], in0=d_rep[:],
                            scalar1=iotaC_t[:, kk:kk + 1], scalar2=None,
                            op0=AT.is_equal)
                        nc.vector.tensor_scalar(
                            out=oheT[:, kk * 128:(kk + 1) * 128], in0=de_rep[:],
                            scalar1=iotaC_t[:, kk:kk + 1], scalar2=None,
                            op0=AT.is_equal)
                    ohlT = sp.tile([128, 128], BF, name="ohlT", tag="ohlT",
                                   bufs=3)
                    nc.vector.tensor_scalar(
                        out=ohlT[:], in0=len_rep[:],
                        scalar1=iotaC_t[:, 0:1], scalar2=None, op0=AT.is_equal)

                    # banded softmax weights [n, tau], then DMA-XBAR transpose
                    m1 = sp.tile([128, K_WIN], F32, name="m1", tag="m1", bufs=2)
                    nc.vector.tensor_scalar(
                        out=m1[:, :W], in0=iotaKr[:, :W],
                        scalar1=dmat_t[:, g:g + 1], scalar2=None, op0=AT.is_ge)
                    m2 = sp.tile([128, K_WIN], F32, name="m2", tag="m2", bufs=2)
                    nc.vector.tensor_scalar(
                        out=m2[:, :W], in0=iotaKr[:, :W],
                        scalar1=dlmat_t[:, g:g + 1], scalar2=None, op0=AT.is_le)
                    e2 = sp.tile([128, K_WIN], F32, name="e2", tag="e2", bufs=2)
                    nc.vector.tensor_tensor(out=e2[:, :W], in0=m1[:, :W],
                                            in1=m2[:, :W], op=AT.mult)
                    eb = sp.tile([128, K_WIN], F32, name="eb", tag="eb", bufs=2)
                    nc.vector.tensor_tensor(out=eb[:, :W], in0=e2[:, :W],
                                            in1=e_rep[:, :W], op=AT.mult)
                    ssum = sp.tile([128, 1], F32, name="ssum", tag="ssum",
                                   bufs=3)
                    nc.vector.tensor_reduce(out=ssum[:], in_=eb[:, :W],
                                            axis=AX.X, op=AT.add)
                    rinv = sp.tile([128, 1], F32, name="rinv", tag="rinv",
                                   bufs=3)
                    nc.vector.reciprocal(rinv[:], ssum[:])
                    wg = sp.tile([128, K_WIN], BF, name="wg", tag="wg", bufs=3)
                    nc.vector.tensor_scalar(out=wg[:, :W], in0=eb[:, :W],
                                            scalar1=rinv[:, 0:1], scalar2=None,
                                            op0=AT.mult)
                    wgT = sp.tile([128, K_WIN], BF, name="wgT", tag="wgT",
                                  bufs=3)
                    for kk in range(KCg):
                        dma(out=wgT[:, kk * 128:(kk + 1) * 128],
                            in_=wg[:, kk * 128:(kk + 1) * 128], transpose=True)

                    # h1[n, h] accumulation (one-hots stationary, N=512),
                    # then DMA-XBAR transpose into h1big[h, n]
                    h1nh = sp.tile([128, HID], BF, name="h1nh", tag="h1nh",
                                   bufs=2)
                    for half in range(2):
                        hs = slice(half * 512, (half + 1) * 512)
                        hp = ps.tile([128, 512], F32, name="psN", tag="psN",
                                     bufs=3)
                        steps = []
                        for kk in range(KCg):
                            ks = slice(kk * 128, (kk + 1) * 128)
                            steps.append((ohT[:, ks], P1[:, c0 + kk, hs]))
                            steps.append((oheT[:, ks], P2[:, c0 + kk, hs]))
                            steps.append((wgT[:, ks], P3[:, c0 + kk, hs]))
                        steps.append((ohlT[:], WBfull[:, hs]))
                        for i, (lhsT, rhs) in enumerate(steps):
                            nc.tensor.matmul(hp[:], lhsT, rhs, start=(i == 0),
                                             stop=(i == len(steps) - 1))
                        nc.scalar.activation(h1nh[:, hs], hp[:], AF.Relu)
                    for k in range(8):
                        dma(out=h1big[:, k, gcol:gcol + 128],
                            in_=h1nh[:, k * 128:(k + 1) * 128], transpose=True)

                    # every 4 groups: span-MLP L2+L3 on the 512-col block
                    if g % 4 == 3:
                        b0 = (g // 4) * 512
                        h2big = sp.tile([128, 8, 512], BF, name="h2big",
                                        tag="h2big", bufs=2)
                        for h2c in range(8):
                            pt = ps.tile([128, 512], F32, name="psA",
                                         tag="psA", bufs=2)
                            for k in range(8):
                                nc.tensor.matmul(
                                    pt[:], w2_t[:, k, h2c * 128:(h2c + 1) * 128],
                                    h1big[:, k, :], start=(k == 0),
                                    stop=(k == 7))
                            nc.scalar.activation(h2big[:, h2c, :], pt[:],
                                                 AF.Relu,
                                                 bias=b2_t[:, h2c:h2c + 1])
                        pt = ps.tile([1, 512], F32, name="psL", tag="psL",
                                     bufs=1)
                        for k in range(8):
                            nc.tensor.matmul(
                                pt[:], w3_t[:, k:k + 1],
                                h2big[:, k, :], start=(k == 0), stop=(k == 7))
                        ob = sp.tile([1, 512], F32, name="ob", tag="ob",
                                     bufs=2)
                        nc.vector.tensor_scalar(out=ob[:], in0=pt[:],
                                                scalar1=float(b3val),
                                                scalar2=None, op0=AT.add)
                        dma(out=scores_p[:, b0:b0 + 512], in_=ob[:])

    if SPLIT_WAITS:
        _split_waits(nc)
    return nc


def _split_waits(nc, max_waits=1):
    """This walrus build rejects instructions carrying >max_waits sem waits
    ("Too many sync wait commands"). Hoist excess waits onto same-engine
    NoOps placed immediately before the instruction — identical semantics
    (engine queues are in-order)."""
    ctr = [0]
    for f in nc.m.functions:
        for blk in f.blocks:
            out = []
            for ins in blk.instructions:
                si = getattr(ins, "sync_info", None)
                if si is not None and si.on_wait and len(si.on_wait) > max_waits:
                    waits = list(si.on_wait)
                    for w in waits[:-max_waits]:
                        ctr[0] += 1
                        nop = mybir.InstNoOp(
                            name=f"I-wsplit-{ctr[0]}", ins=[], outs=[],
                            sync_info=mybir.SyncInfo(on_wait=[w], on_update=[]),
                        )
                        nop.engine = ins.engine
                        out.append(nop)
                    ins.sync_info = mybir.SyncInfo(
                        on_wait=waits[-max_waits:],
                        on_update=list(si.on_update or []),
                    )
                out.append(ins)
            blk.instructions[:] = out
    return ctr[0]


_CACHE = {}
LAST_EXEC_NS = None
TRACE = False


def _install_ntff_shim():
    try:
        import antenv.axon_hooks  # noqa: F401
        return
    except ImportError:
        pass
    try:
        from trn_agent_boot.trn_boot import _ntff_profile_via_ctypes
        hook = _ntff_profile_via_ctypes("/opt/axon/libaxon_pjrt.so")
    except Exception:
        hook = None
    m1 = types.ModuleType("antenv")
    m2 = types.ModuleType("antenv.axon_hooks")
    m2.get_axon_ntff_profile_hook = lambda: hook
    m2.set_axon_ntff_profile_hook = lambda h: None
    m1.axon_hooks = m2
    sys.modules.setdefault("antenv", m1)
    sys.modules["antenv.axon_hooks"] = m2


def _wlay(w, dt):
    """[K, M] -> [128, 8, M] '(ks p) m' layout."""
    w = np.asarray(w, np.float32)
    K, M = w.shape
    return np.ascontiguousarray(
        w.reshape(K // 128, 128, M).transpose(1, 0, 2)).astype(dt)


def _prepare(inputs):
    inp = {k: np.asarray(v) for k, v in inputs.items()}
    ss = inp["span_starts"].astype(np.int64)
    sl = inp["span_lengths"].astype(np.int64)
    plan = _plan(ss, sl)
    T_cap, bases, kcs = plan["T_cap"], plan["bases"], plan["kcs"]
    KC = max(kcs)
    NB = -(-T_cap // 256)
    b3val = float(np.asarray(inp["score_b3"]).reshape(-1)[0])
    ab3val = float(np.asarray(inp["attn_b3"]).reshape(-1)[0])

    key = (T_cap, tuple(bases), tuple(kcs), b3val, ab3val)
    if key not in _CACHE:
        _CACHE[key] = _build(T_cap, bases, kcs, b3val, ab3val)
    nc = _CACHE[key]

    sw1 = inp["score_w1"].astype(np.float32)
    shared = {
        "aw1": _wlay(inp["attn_w1"] * FS, f8e4).reshape(128, -1),
        "aw2": _wlay(inp["attn_w2"] * FS, f8e4).reshape(128, -1),
        "aw3": _wlay(inp["attn_w3"] * FS, f8e4).reshape(128, 8, 1),
        "ab1m": np.ascontiguousarray(
            inp["attn_b1"].astype(np.float32).reshape(8, 128).T) * FS,
        "ab2m": np.ascontiguousarray(
            inp["attn_b2"].astype(np.float32).reshape(8, 128).T) * FS,
        "w1a": _wlay(sw1[0:1024], bf16).reshape(128, -1),
        "w1b": _wlay(sw1[1024:2048], bf16).reshape(128, -1),
        "w1c": _wlay(sw1[2048:3072], bf16).reshape(128, -1),
        "w1d": np.ascontiguousarray(sw1[3072:3092]).astype(bf16),
        "wtT": np.ascontiguousarray(
            inp["width_table"].astype(np.float32).T).astype(bf16),
        "b1r": inp["score_b1"].astype(np.float32).reshape(1, HID).astype(bf16),
        "w2": _wlay(inp["score_w2"], bf16).reshape(128, -1),
        "b2m": np.ascontiguousarray(
            inp["score_b2"].astype(np.float32).reshape(8, 128).T),
        "w3m": _wlay(inp["score_w3"], bf16).reshape(128, 8),
        "iotaK": np.arange(KC * 128, dtype=np.float32).reshape(1, -1),
        "iotaC": np.ascontiguousarray(
            (np.arange(128, dtype=np.float32)[:, None]
             + 128.0 * np.arange(KC, dtype=np.float32)[None, :])),
    }

    states = inp["states"].astype(np.float32)
    embeds = inp["embeds"].astype(np.float32)

    def blocked(xT_pad):
        # [1024, NB*256] -> [128, NB, 8, 256]
        return np.ascontiguousarray(
            xT_pad.reshape(8, 128, NB, 256).transpose(1, 2, 0, 3)
        ).astype(bf16)

    in_maps = []
    for c in range(N_CORES):
        cb = int(plan["core_base"][c])
        stl = np.zeros((D, NB * 256), np.float32)
        eml = np.zeros((D, NB * 256), np.float32)
        hi = min(T, cb + T_cap)
        stl[:, : hi - cb] = states[cb:hi].T
        eml[:, : hi - cb] = embeds[cb:hi].T
        m = dict(shared)
        m["statesTb"] = blocked(stl)
        m["embedsTb"] = blocked(eml)
        d = plan["d"][c].astype(np.float32)
        dl = plan["dl"][c].astype(np.float32)
        ln = plan["ln"][c].astype(np.float32)
        m["dmat"] = np.ascontiguousarray(d.reshape(G, 128).T)
        m["dlmat"] = np.ascontiguousarray(dl.reshape(G, 128).T)
        m["dflat"] = d.reshape(1, C)
        m["deflat"] = dl.reshape(1, C)
        m["lenflat"] = ln.reshape(1, C)
        in_maps.append(m)

    return nc, in_maps, plan


def kernel(**inputs):
    global LAST_EXEC_NS
    from concourse.bass_utils import run_bass_kernel_spmd

    nc, in_maps, plan = _prepare(inputs)
    _install_ntff_shim()
    res = run_bass_kernel_spmd(nc, in_maps, list(range(N_CORES)), trace=TRACE)
    LAST_EXEC_NS = res.exec_time_ns

    out = np.empty(NSPAN, np.float32)
    for c in range(N_CORES):
        out[plan["order"][c * C: (c + 1) * C]] = np.asarray(
            res.results[c]["scores"]).reshape(-1)
    return out.reshape(NSPAN, 1)


# revision 13
# speedup vs baseline: 1.6553x; 1.6553x over previous
"""Trainium2 Bass kernel for nn_MentionScore.

Strategy: sort spans by start, shard 2048 consecutive sorted spans per core.
Each core only touches a ~1.2k-token window of states/embeds. The ragged
gather/softmax/weighted-sum becomes dense matmuls against one-hot / banded
matrices built on-device with iota-compare vector ops. Layer-1 of the span
MLP is algebraically folded:
  h1 = relu(OH_s.T@P1 + OH_e.T@P2 + Wg.T@P3 + onehot(len).T@WB)
with P1=states@W1a, P2=states@W1b, P3=embeds@W1c precomputed per token
(kept in SBUF, group windows 128-aligned) and WB = width_table@W1d + b1.

The attention-logit MLP runs in fp8 (e4m3, weights pre-scaled by 32) with
DoubleRow matmuls; exp(logits) is produced directly by the scalar engine so
the span stage builds the normalized band matrix with two range-compares
against a broadcast exp row (no per-group softmax). Band matrices are
transposed by the DMA XBAR instead of the PE array.
"""

import sys
import types

import numpy as np
import ml_dtypes

import concourse.bass as bass
import concourse.mybir as mybir
from concourse.ap import AP
from concourse.tile import TileContext
from concourse.vector_clock import ScopedClock

BF = mybir.dt.bfloat16
F32 = mybir.dt.float32
F8 = mybir.dt.float8e4
AT = mybir.AluOpType
AF = mybir.ActivationFunctionType
AX = mybir.AxisListType
PM = mybir.MatmulPerfMode
bf16 = ml_dtypes.bfloat16
f8e4 = ml_dtypes.float8_e4m3

N_CORES = 8
T, NSPAN, D, HID, LMAX, WD = 8192, 16384, 1024, 1024, 10, 20
C = NSPAN // N_CORES          # spans per core
G = C // 128                  # 128-span groups per core
FS = 32.0                     # fp8 weight prescale


class PatchedTileContext(TileContext):
    """Workaround: walrus rejects the tail Drain when it carries >1 sem wait
    ("Too many sync wait commands"). Put each wait on its own NoOp instead."""

    def _drain_and_barrier(self, tick_clock, wait_clock):
        nc = self.nc
        drain_inst = nc.sync.drain()
        wait_clock.add_sem_waits(
            drain_inst.ins, ScopedClock({None: tick_clock.global_clock})
        )
        si = drain_inst.ins.sync_info
        if si is not None and si.on_wait is not None and len(si.on_wait) > 1:
            waits = list(si.on_wait)
            drain_inst.ins.sync_info = mybir.SyncInfo(
                on_wait=[waits[0]], on_update=list(si.on_update or [])
            )
            for w in waits[1:]:
                nop = nc.sync.nop()
                nop.ins.sync_info = mybir.SyncInfo(on_wait=[w], on_update=[])

        nc.all_engine_barrier()
        assert self.sems is not None
        popped = nc._tile_sem_poison_stack.pop()
        assert popped is self._sem_poison
        nc.clear_and_free_semaphores(list(self.sems.allocated().values()))
        nc.all_engine_barrier()


def _ceil128(x):
    return int(-(-int(x) // 128) * 128)


def _plan(span_starts, span_lengths):
    """Host-side sharding plan. Returns per-core data + static layout consts."""
    order = np.argsort(span_starts, kind="stable").astype(np.int64)
    ss = span_starts[order].reshape(N_CORES, C).astype(np.int64)
    sl = span_lengths[order].reshape(N_CORES, C).astype(np.int64)
    core_base = ss[:, 0].copy()
    sloc = ss - core_base[:, None]
    eloc = sloc + sl

    T_cap = _ceil128(int(eloc.max()) + 1)
    # 128-aligned shared-across-cores group window bases + per-group k-tiles
    mn = sloc.reshape(N_CORES, G, 128).min(axis=2).min(axis=0)   # [G]
    mx = eloc.reshape(N_CORES, G, 128).max(axis=2).max(axis=0)   # [G]
    bases = (mn // 128) * 128
    kcs = -(-(mx - bases + 1) // 128)
    d = sloc - np.repeat(bases, 128)[None, :]
    assert d.min() >= 0
    assert ((d + sl) <= np.repeat(kcs, 128)[None, :] * 128 - 1).all()

    return {
        "order": order,
        "core_base": core_base,
        "d": d.astype(np.float64),
        "dl": (d + sl).astype(np.float64),
        "ln": sl.astype(np.float64),
        "T_cap": T_cap,
        "bases": [int(b) for b in bases],
        "kcs": [int(k) for k in kcs],
    }


NGROUPS = G
SPLIT_WAITS = True


def _build(T_cap, bases, kcs, b3val, ab3val):
    """Build the single SPMD Bass program (static; shared by all 8 cores)."""
    TC = T_cap // 128
    NCH = TC + 2                      # P chunks incl zero pad
    KC = max(kcs)
    K_WIN = KC * 128
    NB = -(-T_cap // 256)             # 256-token blocks
    T_pad2 = (NCH + 1) * 128
    nc = bass.Bass()

    def par(name, shape, dt):
        return nc.declare_dram_parameter(name, list(shape), dt, isOutput=False)

    statesTb_p = par("statesTb", [128, NB, 8, 256], BF)
    embedsTb_p = par("embedsTb", [128, NB, 8, 256], BF)
    dflat_p = par("dflat", [1, C], F32)
    deflat_p = par("deflat", [1, C], F32)
    lenflat_p = par("lenflat", [1, C], F32)
    aw1_p = par("aw1", [128, 8 * HID], F8)
    aw2_p = par("aw2", [128, 8 * HID], F8)
    aw3_p = par("aw3", [128, 8, 1], F8)
    ab1_p = par("ab1m", [128, 8], F32)
    ab2_p = par("ab2m", [128, 8], F32)
    w1a_p = par("w1a", [128, 8 * HID], BF)
    w1b_p = par("w1b", [128, 8 * HID], BF)
    w1c_p = par("w1c", [128, 8 * HID], BF)
    w1d_p = par("w1d", [WD, HID], BF)
    wtT_p = par("wtT", [WD, LMAX], BF)
    b1r_p = par("b1r", [1, HID], BF)
    w2_p = par("w2", [128, 8 * HID], BF)
    b2_p = par("b2m", [128, 8], F32)
    w3_p = par("w3m", [128, 8], BF)
    iotaC_p = par("iotaC", [128, KC], F32)
    scores_p = nc.declare_dram_parameter("scores", [1, C], F32, isOutput=True)

    with PatchedTileContext(nc) as tc:
        with (
            tc.tile_pool(name="pp", bufs=1) as pp,
            tc.tile_pool(name="ps", bufs=2, space="PSUM") as ps,
            tc.tile_pool(name="dp", bufs=1, space="DRAM") as dp,
        ):
            dma = nc.sync.dma_start

            expa_d = dp.tile([1, T_pad2], F32, name="expa_d", tag="expa_d")
            rr_d = dp.tile([1, C], F32, name="rr_d", tag="rr_d")

            # ---------- persistent tiles ----------
            P1 = pp.tile([128, NCH, HID], BF, name="P1", tag="P1")
            P2 = pp.tile([128, NCH, HID], BF, name="P2", tag="P2")
            P3 = pp.tile([128, NCH, HID], BF, name="P3", tag="P3")
            Pmats = (P1, P2, P3)
            w2_t = pp.tile([128, 8, HID], BF, name="w2", tag="w2")
            w3_t = pp.tile([128, 8], BF, name="w3", tag="w3")
            b2_t = pp.tile([128, 8], F32, name="b2", tag="b2")
            WBpad = pp.tile([128, 8, 128], BF, name="WBpad", tag="WBpad")
            iotaC_t = pp.tile([128, KC], F32, name="iotaC", tag="iotaC")
            ones128 = pp.tile([128, 1], BF, name="ones128", tag="ones128")

            with tc.tile_pool(name="tk", bufs=1) as tk:
                # first DMA wave: exactly what block 0 of the token stage
                # needs, split across queues (round-robin -> parallel)
                aw1_t = tk.tile([128, 8, HID], F8, name="aw1", tag="aw1")
                for q in range(4):
                    dma(out=aw1_t[:, 2 * q:2 * q + 2, :],
                        in_=aw1_p[:, 2 * q * HID:(2 * q + 2) * HID])

                sTb = [None] * NB
                eTb = [None] * NB

                def load_block(b):
                    n0 = b * 256
                    nw = min(256, T_cap - n0)
                    sTb[b] = tk.tile([128, 8, 256], BF, name=f"sTb",
                                     tag="sTb", bufs=3)
                    dma(out=sTb[b][:, 0:4, :nw], in_=statesTb_p[:, b, 0:4, :nw])
                    dma(out=sTb[b][:, 4:8, :nw], in_=statesTb_p[:, b, 4:8, :nw])
                    eTb[b] = tk.tile([128, 8, 256], BF, name=f"eTb",
                                     tag="eTb", bufs=3)
                    dma(out=eTb[b][:, 0:4, :nw], in_=embedsTb_p[:, b, 0:4, :nw])
                    dma(out=eTb[b][:, 4:8, :nw], in_=embedsTb_p[:, b, 4:8, :nw])

                load_block(0)
                ab1_t = tk.tile([128, 8], F32, name="ab1", tag="ab1")
                dma(out=ab1_t[:], in_=ab1_p[:])
                ab2_t = tk.tile([128, 8], F32, name="ab2", tag="ab2")
                dma(out=ab2_t[:], in_=ab2_p[:])
                aw3_t = tk.tile([128, 8, 1], F8, name="aw3", tag="aw3")
                dma(out=aw3_t[:], in_=aw3_p[:])
                dma(out=iotaC_t[:], in_=iotaC_p[:])

                # second wave: weights for the rest of the pipeline
                aw2_t = tk.tile([128, 8, HID], F8, name="aw2", tag="aw2")
                for q in range(2):
                    dma(out=aw2_t[:, 4 * q:4 * q + 4, :],
                        in_=aw2_p[:, 4 * q * HID:(4 * q + 4) * HID])
                w1_t = []
                for i, p_ in enumerate((w1a_p, w1b_p, w1c_p)):
                    t = tk.tile([128, 8, HID], BF, name=f"w1_{i}", tag=f"w1_{i}")
                    for q in range(2):
                        dma(out=t[:, 4 * q:4 * q + 4, :],
                            in_=p_[:, 4 * q * HID:(4 * q + 4) * HID])
                    w1_t.append(t)
                load_block(1)
                for q in range(2):
                    dma(out=w2_t[:, 4 * q:4 * q + 4, :],
                        in_=w2_p[:, 4 * q * HID:(4 * q + 4) * HID])
                dma(out=w3_t[:], in_=w3_p[:])
                dma(out=b2_t[:], in_=b2_p[:])
                nc.vector.memset(ones128[:], 1.0)
                wtT_t = tk.tile([WD, 16], BF, name="wtT", tag="wtT")
                nc.vector.memset(wtT_t[:], 0.0)
                dma(out=wtT_t[:, :LMAX], in_=wtT_p[:])
                w1d_t = tk.tile([WD, HID], BF, name="w1d", tag="w1d")
                dma(out=w1d_t[:], in_=w1d_p[:])
                b1r_t = tk.tile([1, HID], BF, name="b1r", tag="b1r")
                dma(out=b1r_t[:], in_=b1r_p[:])
                ones16_t = tk.tile([1, 16], BF, name="ones16", tag="ones16")
                nc.vector.memset(ones16_t[:], 1.0)

                # zero-fill upper P chunks + expa pad once
                nc.vector.memset(P1[:, TC:, :], 0.0)
                nc.vector.memset(P2[:, TC:, :], 0.0)
                nc.vector.memset(P3[:, TC:, :], 0.0)
                zpad = tk.tile([1, T_pad2 - T_cap], F32, name="zpad", tag="zpad")
                nc.vector.memset(zpad[:], 0.0)
                dma(out=expa_d[0:1, T_cap:], in_=zpad[:])

                # WBpad = [width_table@W1d + b1 ; 0] as [128(k), 8(hc), 128]
                nc.vector.memset(WBpad[:], 0.0)
                for h0 in (0, 512):
                    wbp = ps.tile([16, 512], F32, name="wbp", tag="wbp", bufs=1)
                    nc.tensor.matmul(wbp[:], wtT_t[:], w1d_t[:, h0:h0 + 512],
                                     start=True, stop=False)
                    nc.tensor.matmul(wbp[:], ones16_t[:], b1r_t[:, h0:h0 + 512],
                                     start=False, stop=True)
                    for cq in range(4):
                        hc = h0 // 128 + cq
                        nc.scalar.copy(WBpad[0:16, hc, :],
                                       wbp[:, cq * 128:(cq + 1) * 128])

                # ---------- token pipeline ----------
                for b in range(NB):
                    n0 = b * 256
                    nw = min(256, T_cap - n0)
                    if b + 2 < NB:
                        load_block(b + 2)
                    sT8 = tk.tile([128, 8, 256], F8, name="sT8", tag="sT8",
                                  bufs=2)
                    nc.vector.tensor_copy(out=sT8[:, :, :nw],
                                          in_=sTb[b][:, :, :nw])
                    # attn l1 (fp8 DoubleRow)
                    h1a = tk.tile([128, 8, 256], F8, name="h1a", tag="h1a",
                                  bufs=2)
                    for hc in range(8):
                        pt = ps.tile([128, 512], F32, name="psA", tag="psA",
                                     bufs=2)
                        for jp in range(4):
                            nc.tensor.matmul(
                                pt[:, :nw],
                                aw1_t[:, 2 * jp:2 * jp + 2,
                                      hc * 128:(hc + 1) * 128],
                                sT8[:, 2 * jp:2 * jp + 2, :nw],
                                start=(jp == 0), stop=(jp == 3),
                                perf_mode=PM.DoubleRow)
                        nc.scalar.activation(h1a[:, hc, :nw], pt[:, :nw],
                                             AF.Relu, bias=ab1_t[:, hc:hc + 1])
                    # attn l2
                    h2a = tk.tile([128, 8, 256], F8, name="h2a", tag="h2a",
                                  bufs=2)
                    for hc in range(8):
                        pt = ps.tile([128, 512], F32, name="psA", tag="psA",
                                     bufs=2)
                        for jp in range(4):
                            nc.tensor.matmul(
                                pt[:, :nw],
                                aw2_t[:, 2 * jp:2 * jp + 2,
                                      hc * 128:(hc + 1) * 128],
                                h1a[:, 2 * jp:2 * jp + 2, :nw],
                                start=(jp == 0), stop=(jp == 3),
                                perf_mode=PM.DoubleRow)
                        nc.scalar.activation(h2a[:, hc, :nw], pt[:, :nw],
                                             AF.Relu, bias=ab2_t[:, hc:hc + 1],
                                             scale=1.0 / FS)
                    # attn l3 -> exp(logits)
                    pt = ps.tile([1, 512], F32, name="psL", tag="psL", bufs=1)
                    for k in range(8):
                        nc.tensor.matmul(
                            pt[:, :nw],
                            aw3_t[:, k, :],
                            h2a[:, k, :nw],
                            start=(k == 0), stop=(k == 7))
                    expb = tk.tile([1, 256], F32, name="expb", tag="expb",
                                   bufs=2)
                    nc.scalar.activation(expb[:, :nw], pt[:, :nw], AF.Exp,
                                         bias=float(ab3val),
                                         scale=1.0 / (FS * FS))
                    dma(out=expa_d[0:1, n0:n0 + nw], in_=expb[:, :nw])

                    # projections P1/P2/P3 (bf16)
                    for pi in range(3):
                        src = sTb[b] if pi < 2 else eTb[b]
                        for j in range(nw // 128):
                            ch = (n0 + j * 128) // 128
                            for h0 in (0, 512):
                                pt = ps.tile([128, 512], F32, name="psA",
                                             tag="psA", bufs=2)
                                for k in range(8):
                                    nc.tensor.matmul(
                                        pt[:],
                                        src[:, k, j * 128:(j + 1) * 128],
                                        w1_t[pi][:, k, h0:h0 + 512],
                                        start=(k == 0), stop=(k == 7))
                                nc.vector.tensor_copy(
                                    out=Pmats[pi][:, ch, h0:h0 + 512],
                                    in_=pt[:])

            # ---------- span stage ----------
            with tc.tile_pool(name="sp", bufs=1) as sp:
                h1big = h2big = None
                for g in range(NGROUPS):
                    KCg = kcs[g]
                    c0 = bases[g] // 128
                    W = KCg * 128
                    gcol = (g % 4) * 128
                    if g % 4 == 0:
                        h1big = sp.tile([128, 8, 512], BF, name="h1big",
                                        tag="h1big", bufs=2)

                    d_rep = sp.tile([128, 128], F32, name="d_rep",
                                    tag="d_rep", bufs=3)
                    dma(out=d_rep[:],
                        in_=dflat_p[:, g * 128:(g + 1) * 128]
                        .partition_broadcast(128))
                    de_rep = sp.tile([128, 128], F32, name="de_rep",
                                     tag="de_rep", bufs=3)
                    dma(out=de_rep[:],
                        in_=deflat_p[:, g * 128:(g + 1) * 128]
                        .partition_broadcast(128))
                    len_rep = sp.tile([128, 128], F32, name="len_rep",
                                      tag="len_rep", bufs=3)
                    dma(out=len_rep[:],
                        in_=lenflat_p[:, g * 128:(g + 1) * 128]
                        .partition_broadcast(128))
                    e_col = sp.tile([128, KC], F32, name="e_col",
                                    tag="e_col", bufs=3)
                    dma(out=e_col[:, :KCg],
                        in_=AP(tensor=expa_d.tensor, offset=bases[g],
                               ap=[[1, 128], [128, KCg]]))

                    # one-hots [tau, n]
                    ohT = sp.tile([128, K_WIN], BF, name="ohT", tag="ohT",
                                  bufs=3)
                    oheT = sp.tile([128, K_WIN], BF, name="oheT", tag="oheT",
                                   bufs=3)
                    for kk in range(KCg):
                        nc.vector.tensor_scalar(
                            out=ohT[:, kk * 128:(kk + 1) * 128], in0=d_rep[:],
                            scalar1=iotaC_t[:, kk:kk + 1], scalar2=None,
                            op0=AT.is_equal)
                        nc.vector.tensor_scalar(
                            out=oheT[:, kk * 128:(kk + 1) * 128], in0=de_rep[:],
                            scalar1=iotaC_t[:, kk:kk + 1], scalar2=None,
                            op0=AT.is_equal)
                    ohlT = sp.tile([128, 128], BF, name="ohlT", tag="ohlT",
                                   bufs=3)
                    nc.vector.tensor_scalar(
                        out=ohlT[:], in0=len_rep[:],
                        scalar1=iotaC_t[:, 0:1], scalar2=None, op0=AT.is_equal)

                    # banded exp weights built directly as [tau, n]
                    eb = sp.tile([128, K_WIN], BF, name="eb", tag="eb", bufs=3)
                    x1 = sp.tile([128, 128], BF, name="x1", tag="x1", bufs=2)
                    x2 = sp.tile([128, 128], BF, name="x2", tag="x2", bufs=2)
                    for kk in range(KCg):
                        nc.vector.tensor_scalar(
                            out=x1[:], in0=d_rep[:],
                            scalar1=iotaC_t[:, kk:kk + 1], scalar2=None,
                            op0=AT.is_le)
                        nc.vector.tensor_scalar(
                            out=x2[:], in0=de_rep[:],
                            scalar1=iotaC_t[:, kk:kk + 1],
                            scalar2=e_col[:, kk:kk + 1],
                            op0=AT.is_ge, op1=AT.mult)
                        nc.vector.tensor_tensor(
                            out=eb[:, kk * 128:(kk + 1) * 128], in0=x1[:],
                            in1=x2[:], op=AT.mult)
                    sps = ps.tile([1, 128], F32, name="psS", tag="psS", bufs=1)
                    for kk in range(KCg):
                        nc.tensor.matmul(sps[:], ones128[:],
                                         eb[:, kk * 128:(kk + 1) * 128],
                                         start=(kk == 0), stop=(kk == KCg - 1))
                    srow = sp.tile([1, 128], F32, name="srow", tag="srow",
                                   bufs=3)
                    nc.vector.reciprocal(srow[:], sps[:])
                    dma(out=rr_d[0:1, g * 128:(g + 1) * 128], in_=srow[:])
                    rinv_rep = sp.tile([128, 128], F32, name="rinv_rep",
                                       tag="rinv_rep", bufs=3)
                    dma(out=rinv_rep[:],
                        in_=rr_d[0:1, g * 128:(g + 1) * 128]
                        .partition_broadcast(128))
                    wgT = sp.tile([128, K_WIN], BF, name="wgT", tag="wgT",
                                  bufs=3)
                    for kk in range(KCg):
                        nc.vector.tensor_tensor(
                            out=wgT[:, kk * 128:(kk + 1) * 128],
                            in0=eb[:, kk * 128:(kk + 1) * 128],
                            in1=rinv_rep[:], op=AT.mult)

                    # h1^T[h, n] accumulation
                    for hc in range(8):
                        hp = ps.tile([128, 128], F32, name="psH", tag="psH",
                                     bufs=2)
                        hs = slice(hc * 128, (hc + 1) * 128)
                        steps = []
                        for kk in range(KCg):
                            ks = slice(kk * 128, (kk + 1) * 128)
                            steps.append((P1[:, c0 + kk, hs], ohT[:, ks]))
                            steps.append((P2[:, c0 + kk, hs], oheT[:, ks]))
                        steps.append((WBpad[:, hc, :], ohlT[:]))
                        for kk in range(KCg):
                            ks = slice(kk * 128, (kk + 1) * 128)
                            steps.append((P3[:, c0 + kk, hs], wgT[:, ks]))
                        for i, (lhsT, rhs) in enumerate(steps):
                            nc.tensor.matmul(hp[:], lhsT, rhs, start=(i == 0),
                                             stop=(i == len(steps) - 1))
                        nc.vector.tensor_scalar(
                            out=h1big[:, hc, gcol:gcol + 128], in0=hp[:],
                            scalar1=0.0, scalar2=None, op0=AT.max)

                    # every 4 groups: span-MLP L2+L3 on the 512-col block
                    if g % 4 == 3:
                        b0 = (g // 4) * 512
                        h2big = sp.tile([128, 8, 512], BF, name="h2big",
                                        tag="h2big", bufs=2)
                        for h2c in range(8):
                            pt = ps.tile([128, 512], F32, name="psA",
                                         tag="psA", bufs=2)
                            for k in range(8):
                                nc.tensor.matmul(
                                    pt[:], w2_t[:, k, h2c * 128:(h2c + 1) * 128],
                                    h1big[:, k, :], start=(k == 0),
                                    stop=(k == 7))
                            nc.scalar.activation(h2big[:, h2c, :], pt[:],
                                                 AF.Relu,
                                                 bias=b2_t[:, h2c:h2c + 1])
                        pt = ps.tile([1, 512], F32, name="psL", tag="psL",
                                     bufs=1)
                        for k in range(8):
                            nc.tensor.matmul(
                                pt[:], w3_t[:, k:k + 1],
                                h2big[:, k, :], start=(k == 0), stop=(k == 7))
                        ob = sp.tile([1, 512], F32, name="ob", tag="ob",
                                     bufs=2)
                        nc.vector.tensor_scalar(out=ob[:], in0=pt[:],
                                                scalar1=float(b3val),
                                                scalar2=None, op0=AT.add)
                        dma(out=scores_p[:, b0:b0 + 512], in_=ob[:])

    if SPLIT_WAITS:
        _split_waits(nc)
    return nc


def _split_waits(nc, max_waits=1):
    """This walrus build rejects instructions carrying >max_waits sem waits
    ("Too many sync wait commands"). Hoist excess waits onto same-engine
    NoOps placed immediately before the instruction — identical semantics
    (engine queues are in-order)."""
    ctr = [0]
    for f in nc.m.functions:
        for blk in f.blocks:
            out = []
            for ins in blk.instructions:
                si = getattr(ins, "sync_info", None)
                if si is not None and si.on_wait and len(si.on_wait) > max_waits:
                    waits = list(si.on_wait)
                    for w in waits[:-max_waits]:
                        ctr[0] += 1
                        nop = mybir.InstNoOp(
                            name=f"I-wsplit-{ctr[0]}", ins=[], outs=[],
                            sync_info=mybir.SyncInfo(on_wait=[w], on_update=[]),
                        )
                        nop.engine = ins.engine
                        out.append(nop)
                    ins.sync_info = mybir.SyncInfo(
                        on_wait=waits[-max_waits:],
                        on_update=list(si.on_update or []),
                    )
                out.append(ins)
            blk.instructions[:] = out
    return ctr[0]


_CACHE = {}
LAST_EXEC_NS = None
TRACE = False


def _install_ntff_shim():
    try:
        import antenv.axon_hooks  # noqa: F401
        return
    except ImportError:
        pass
    try:
        from trn_agent_boot.trn_boot import _ntff_profile_via_ctypes
        hook = _ntff_profile_via_ctypes("/opt/axon/libaxon_pjrt.so")
    except Exception:
        hook = None
    m1 = types.ModuleType("antenv")
    m2 = types.ModuleType("antenv.axon_hooks")
    m2.get_axon_ntff_profile_hook = lambda: hook
    m2.set_axon_ntff_profile_hook = lambda h: None
    m1.axon_hooks = m2
    sys.modules.setdefault("antenv", m1)
    sys.modules["antenv.axon_hooks"] = m2


def _wlay(w, dt):
    """[K, M] -> [128, 8, M] '(ks p) m' layout."""
    w = np.asarray(w, np.float32)
    K, M = w.shape
    return np.ascontiguousarray(
        w.reshape(K // 128, 128, M).transpose(1, 0, 2)).astype(dt)


def _prepare(inputs):
    inp = {k: np.asarray(v) for k, v in inputs.items()}
    ss = inp["span_starts"].astype(np.int64)
    sl = inp["span_lengths"].astype(np.int64)
    plan = _plan(ss, sl)
    T_cap, bases, kcs = plan["T_cap"], plan["bases"], plan["kcs"]
    KC = max(kcs)
    NB = -(-T_cap // 256)
    b3val = float(np.asarray(inp["score_b3"]).reshape(-1)[0])
    ab3val = float(np.asarray(inp["attn_b3"]).reshape(-1)[0])

    key = (T_cap, tuple(bases), tuple(kcs), b3val, ab3val)
    if key not in _CACHE:
        _CACHE[key] = _build(T_cap, bases, kcs, b3val, ab3val)
    nc = _CACHE[key]

    sw1 = inp["score_w1"].astype(np.float32)
    shared = {
        "aw1": _wlay(inp["attn_w1"] * FS, f8e4).reshape(128, -1),
        "aw2": _wlay(inp["attn_w2"] * FS, f8e4).reshape(128, -1),
        "aw3": _wlay(inp["attn_w3"] * FS, f8e4).reshape(128, 8, 1),
        "ab1m": np.ascontiguousarray(
            inp["attn_b1"].astype(np.float32).reshape(8, 128).T) * FS,
        "ab2m": np.ascontiguousarray(
            inp["attn_b2"].astype(np.float32).reshape(8, 128).T) * FS,
        "w1a": _wlay(sw1[0:1024], bf16).reshape(128, -1),
        "w1b": _wlay(sw1[1024:2048], bf16).reshape(128, -1),
        "w1c": _wlay(sw1[2048:3072], bf16).reshape(128, -1),
        "w1d": np.ascontiguousarray(sw1[3072:3092]).astype(bf16),
        "wtT": np.ascontiguousarray(
            inp["width_table"].astype(np.float32).T).astype(bf16),
        "b1r": inp["score_b1"].astype(np.float32).reshape(1, HID).astype(bf16),
        "w2": _wlay(inp["score_w2"], bf16).reshape(128, -1),
        "b2m": np.ascontiguousarray(
            inp["score_b2"].astype(np.float32).reshape(8, 128).T),
        "w3m": _wlay(inp["score_w3"], bf16).reshape(128, 8),
        "iotaC": np.ascontiguousarray(
            (np.arange(128, dtype=np.float32)[:, None]
             + 128.0 * np.arange(KC, dtype=np.float32)[None, :])),
    }

    states = inp["states"].astype(np.float32)
    embeds = inp["embeds"].astype(np.float32)

    def blocked(xT_pad):
        # [1024, NB*256] -> [128, NB, 8, 256]
        return np.ascontiguousarray(
            xT_pad.reshape(8, 128, NB, 256).transpose(1, 2, 0, 3)
        ).astype(bf16)

    in_maps = []
    for c in range(N_CORES):
        cb = int(plan["core_base"][c])
        stl = np.zeros((D, NB * 256), np.float32)
        eml = np.zeros((D, NB * 256), np.float32)
        hi = min(T, cb + T_cap)
        stl[:, : hi - cb] = states[cb:hi].T
        eml[:, : hi - cb] = embeds[cb:hi].T
        m = dict(shared)
        m["statesTb"] = blocked(stl)
        m["embedsTb"] = blocked(eml)
        d = plan["d"][c].astype(np.float32)
        dl = plan["dl"][c].astype(np.float32)
        ln = plan["ln"][c].astype(np.float32)
        m["dflat"] = d.reshape(1, C)
        m["deflat"] = dl.reshape(1, C)
        m["lenflat"] = ln.reshape(1, C)
        in_maps.append(m)

    return nc, in_maps, plan


def kernel(**inputs):
    global LAST_EXEC_NS
    from concourse.bass_utils import run_bass_kernel_spmd

    nc, in_maps, plan = _prepare(inputs)
    _install_ntff_shim()
    res = run_bass_kernel_spmd(nc, in_maps, list(range(N_CORES)), trace=TRACE)
    LAST_EXEC_NS = res.exec_time_ns

    out = np.empty(NSPAN, np.float32)
    for c in range(N_CORES):
        out[plan["order"][c * C: (c + 1) * C]] = np.asarray(
            res.results[c]["scores"]).reshape(-1)
    return out.reshape(NSPAN, 1)


# revision 17
# speedup vs baseline: 1.7898x; 1.0812x over previous
"""Trainium2 Bass kernel for nn_MentionScore.

Strategy: sort spans by start, shard 2048 consecutive sorted spans per core.
Each core only touches a ~1.2k-token window of states/embeds. The ragged
gather/softmax/weighted-sum becomes dense matmuls against one-hot / banded
matrices built on-device with iota-compare vector ops. Layer-1 of the span
MLP is algebraically folded:
  h1 = relu(OH_s.T@P1 + OH_e.T@P2 + Wg.T@P3 + onehot(len).T@WB)
with P1=states@W1a, P2=states@W1b, P3=embeds@W1c precomputed per token
(kept in SBUF, group windows 128-aligned) and WB = width_table@W1d + b1.

The attention-logit MLP runs in fp8 (e4m3, weights pre-scaled by 32) with
DoubleRow matmuls; exp(logits) is produced directly by the scalar engine so
the span stage builds the normalized band matrix with two range-compares
against a broadcast exp row (no per-group softmax). Band matrices are
transposed by the DMA XBAR instead of the PE array.
"""

import sys
import types

import numpy as np
import ml_dtypes

import concourse.bass as bass
import concourse.mybir as mybir
from concourse.ap import AP
from concourse.tile import TileContext
from concourse.vector_clock import ScopedClock

BF = mybir.dt.bfloat16
F32 = mybir.dt.float32
F8 = mybir.dt.float8e4
AT = mybir.AluOpType
AF = mybir.ActivationFunctionType
AX = mybir.AxisListType
PM = mybir.MatmulPerfMode
bf16 = ml_dtypes.bfloat16
f8e4 = ml_dtypes.float8_e4m3

N_CORES = 8
T, NSPAN, D, HID, LMAX, WD = 8192, 16384, 1024, 1024, 10, 20
C = NSPAN // N_CORES          # spans per core
G = C // 128                  # 128-span groups per core
FS = 32.0                     # fp8 weight prescale


class PatchedTileContext(TileContext):
    """Workaround: walrus rejects the tail Drain when it carries >1 sem wait
    ("Too many sync wait commands"). Put each wait on its own NoOp instead."""

    def _drain_and_barrier(self, tick_clock, wait_clock):
        nc = self.nc
        drain_inst = nc.sync.drain()
        wait_clock.add_sem_waits(
            drain_inst.ins, ScopedClock({None: tick_clock.global_clock})
        )
        si = drain_inst.ins.sync_info
        if si is not None and si.on_wait is not None and len(si.on_wait) > 1:
            waits = list(si.on_wait)
            drain_inst.ins.sync_info = mybir.SyncInfo(
                on_wait=[waits[0]], on_update=list(si.on_update or [])
            )
            for w in waits[1:]:
                nop = nc.sync.nop()
                nop.ins.sync_info = mybir.SyncInfo(on_wait=[w], on_update=[])

        nc.all_engine_barrier()
        assert self.sems is not None
        popped = nc._tile_sem_poison_stack.pop()
        assert popped is self._sem_poison
        nc.clear_and_free_semaphores(list(self.sems.allocated().values()))
        nc.all_engine_barrier()


def _ceil128(x):
    return int(-(-int(x) // 128) * 128)


def _plan(span_starts, span_lengths):
    """Host-side sharding plan. Returns per-core data + static layout consts."""
    order = np.argsort(span_starts, kind="stable").astype(np.int64)
    ss = span_starts[order].reshape(N_CORES, C).astype(np.int64)
    sl = span_lengths[order].reshape(N_CORES, C).astype(np.int64)
    core_base = ss[:, 0].copy()
    sloc = ss - core_base[:, None]
    eloc = sloc + sl

    T_cap = _ceil128(int(eloc.max()) + 1)
    # 128-aligned shared-across-cores group window bases + per-group k-tiles
    mn = sloc.reshape(N_CORES, G, 128).min(axis=2).min(axis=0)   # [G]
    mx = eloc.reshape(N_CORES, G, 128).max(axis=2).max(axis=0)   # [G]
    bases = (mn // 128) * 128
    kcs = -(-(mx - bases + 1) // 128)
    d = sloc - np.repeat(bases, 128)[None, :]
    assert d.min() >= 0
    assert ((d + sl) <= np.repeat(kcs, 128)[None, :] * 128 - 1).all()

    return {
        "order": order,
        "core_base": core_base,
        "d": d.astype(np.float64),
        "dl": (d + sl).astype(np.float64),
        "ln": sl.astype(np.float64),
        "T_cap": T_cap,
        "bases": [int(b) for b in bases],
        "kcs": [int(k) for k in kcs],
    }


NGROUPS = G
SPLIT_WAITS = True


def _build(T_cap, bases, kcs, b3val, ab3val):
    """Build the single SPMD Bass program (static; shared by all 8 cores)."""
    TC = T_cap // 128
    NCH = TC + 2                      # P chunks incl zero pad
    KC = max(kcs)
    K_WIN = KC * 128
    NB = -(-T_cap // 256)             # 256-token blocks
    T_pad2 = (NCH + 1) * 128
    nc = bass.Bass()

    def par(name, shape, dt):
        return nc.declare_dram_parameter(name, list(shape), dt, isOutput=False)

    statesTb_p = par("statesTb", [128, NB, 8, 256], BF)
    statesT8b_p = par("statesT8b", [128, NB, 8, 256], F8)
    embedsTb_p = par("embedsTb", [128, NB, 8, 256], BF)
    dflat_p = par("dflat", [1, C], F32)
    deflat_p = par("deflat", [1, C], F32)
    lenflat_p = par("lenflat", [1, C], F32)
    aw1_p = par("aw1", [128, 8 * HID], F8)
    aw2_p = par("aw2", [128, 8 * HID], F8)
    aw3_p = par("aw3", [128, 8, 1], F8)
    ab1_p = par("ab1m", [128, 8], F32)
    ab2_p = par("ab2m", [128, 8], F32)
    w1a_p = par("w1a", [128, 8 * HID], BF)
    w1b_p = par("w1b", [128, 8 * HID], BF)
    w1c_p = par("w1c", [128, 8 * HID], BF)
    w1d_p = par("w1d", [WD, HID], BF)
    wtT_p = par("wtT", [WD, LMAX], BF)
    b1r_p = par("b1r", [1, HID], BF)
    w2_p = par("w2", [128, 8 * HID], BF)
    b2_p = par("b2m", [128, 8], F32)
    w3_p = par("w3m", [128, 8], BF)
    iotaC_p = par("iotaC", [128, KC], F32)
    scores_p = nc.declare_dram_parameter("scores", [1, C], F32, isOutput=True)

    with PatchedTileContext(nc) as tc:
        with (
            tc.tile_pool(name="pp", bufs=1) as pp,
            tc.tile_pool(name="ps", bufs=2, space="PSUM") as ps,
            tc.tile_pool(name="dp", bufs=1, space="DRAM") as dp,
        ):
            dma = nc.sync.dma_start

            expa_d = dp.tile([1, T_pad2], F32, name="expa_d", tag="expa_d")
            rr_d = dp.tile([1, C], F32, name="rr_d", tag="rr_d")

            # ---------- persistent tiles ----------
            P1 = pp.tile([128, NCH, HID], BF, name="P1", tag="P1")
            P2 = pp.tile([128, NCH, HID], BF, name="P2", tag="P2")
            P3 = pp.tile([128, NCH, HID], BF, name="P3", tag="P3")
            Pmats = (P1, P2, P3)
            w2_t = pp.tile([128, 8, HID], BF, name="w2", tag="w2")
            w3_t = pp.tile([128, 8], BF, name="w3", tag="w3")
            b2_t = pp.tile([128, 8], F32, name="b2", tag="b2")
            WBpad = pp.tile([128, 8, 128], BF, name="WBpad", tag="WBpad")
            iotaC_t = pp.tile([128, KC], F32, name="iotaC", tag="iotaC")
            ones128 = pp.tile([128, 1], BF, name="ones128", tag="ones128")

            with tc.tile_pool(name="tk", bufs=1) as tk:
                # first DMA wave: exactly what block 0 of the token stage
                # needs, split across queues (round-robin -> parallel)
                aw1_t = tk.tile([128, 8, HID], F8, name="aw1", tag="aw1")
                for q in range(4):
                    dma(out=aw1_t[:, 2 * q:2 * q + 2, :],
                        in_=aw1_p[:, 2 * q * HID:(2 * q + 2) * HID])

                sTb = [None] * NB
                eTb = [None] * NB
                sT8l = [None] * NB

                def load_block(b):
                    n0 = b * 256
                    nw = min(256, T_cap - n0)
                    sT8l[b] = tk.tile([128, 8, 256], F8, name="sT8",
                                      tag="sT8", bufs=3)
                    dma(out=sT8l[b][:, :, :nw], in_=statesT8b_p[:, b, :, :nw])
                    sTb[b] = tk.tile([128, 8, 256], BF, name=f"sTb",
                                     tag="sTb", bufs=3)
                    dma(out=sTb[b][:, 0:4, :nw], in_=statesTb_p[:, b, 0:4, :nw])
                    dma(out=sTb[b][:, 4:8, :nw], in_=statesTb_p[:, b, 4:8, :nw])
                    eTb[b] = tk.tile([128, 8, 256], BF, name=f"eTb",
                                     tag="eTb", bufs=3)
                    dma(out=eTb[b][:, 0:4, :nw], in_=embedsTb_p[:, b, 0:4, :nw])
                    dma(out=eTb[b][:, 4:8, :nw], in_=embedsTb_p[:, b, 4:8, :nw])

                load_block(0)
                ab1_t = tk.tile([128, 8], F32, name="ab1", tag="ab1")
                dma(out=ab1_t[:], in_=ab1_p[:])
                ab2_t = tk.tile([128, 8], F32, name="ab2", tag="ab2")
                dma(out=ab2_t[:], in_=ab2_p[:])
                aw3_t = tk.tile([128, 8, 1], F8, name="aw3", tag="aw3")
                dma(out=aw3_t[:], in_=aw3_p[:])
                dma(out=iotaC_t[:], in_=iotaC_p[:])

                # second wave: weights for the rest of the pipeline
                aw2_t = tk.tile([128, 8, HID], F8, name="aw2", tag="aw2")
                for q in range(2):
                    dma(out=aw2_t[:, 4 * q:4 * q + 4, :],
                        in_=aw2_p[:, 4 * q * HID:(4 * q + 4) * HID])
                w1_t = []
                for i, p_ in enumerate((w1a_p, w1b_p, w1c_p)):
                    t = tk.tile([128, 8, HID], BF, name=f"w1_{i}", tag=f"w1_{i}")
                    for q in range(2):
                        dma(out=t[:, 4 * q:4 * q + 4, :],
                            in_=p_[:, 4 * q * HID:(4 * q + 4) * HID])
                    w1_t.append(t)
                load_block(1)
                for q in range(2):
                    dma(out=w2_t[:, 4 * q:4 * q + 4, :],
                        in_=w2_p[:, 4 * q * HID:(4 * q + 4) * HID])
                dma(out=w3_t[:], in_=w3_p[:])
                dma(out=b2_t[:], in_=b2_p[:])
                nc.vector.memset(ones128[:], 1.0)
                wtT_t = tk.tile([WD, 16], BF, name="wtT", tag="wtT")
                nc.vector.memset(wtT_t[:], 0.0)
                dma(out=wtT_t[:, :LMAX], in_=wtT_p[:])
                w1d_t = tk.tile([WD, HID], BF, name="w1d", tag="w1d")
                dma(out=w1d_t[:], in_=w1d_p[:])
                b1r_t = tk.tile([1, HID], BF, name="b1r", tag="b1r")
                dma(out=b1r_t[:], in_=b1r_p[:])
                ones16_t = tk.tile([1, 16], BF, name="ones16", tag="ones16")
                nc.vector.memset(ones16_t[:], 1.0)

                # zero-fill upper P chunks + expa pad (gpsimd: off the
                # vector/scalar critical path)
                nc.gpsimd.memset(P1[:, TC:, :], 0.0)
                nc.gpsimd.memset(P2[:, TC:, :], 0.0)
                nc.gpsimd.memset(P3[:, TC:, :], 0.0)
                zpad = tk.tile([1, T_pad2 - T_cap], F32, name="zpad", tag="zpad")
                nc.gpsimd.memset(zpad[:], 0.0)
                dma(out=expa_d[0:1, T_cap:], in_=zpad[:])
                nc.gpsimd.memset(WBpad[:], 0.0)

                # ---------- token pipeline ----------
                for b in range(NB):
                    if b == 1:
                        # WBpad = [width_table@W1d + b1 ; 0] as [128, 8, 128]
                        for h0 in (0, 512):
                            wbp = ps.tile([16, 512], F32, name="wbp",
                                          tag="wbp", bufs=1)
                            nc.tensor.matmul(wbp[:], wtT_t[:],
                                             w1d_t[:, h0:h0 + 512],
                                             start=True, stop=False)
                            nc.tensor.matmul(wbp[:], ones16_t[:],
                                             b1r_t[:, h0:h0 + 512],
                                             start=False, stop=True)
                            for cq in range(4):
                                hc = h0 // 128 + cq
                                nc.scalar.copy(WBpad[0:16, hc, :],
                                               wbp[:, cq * 128:(cq + 1) * 128])
                    n0 = b * 256
                    nw = min(256, T_cap - n0)
                    if b + 2 < NB:
                        load_block(b + 2)
                    sT8 = sT8l[b]
                    # attn l1 (fp8 DoubleRow)
                    h1a = tk.tile([128, 8, 256], F8, name="h1a", tag="h1a",
                                  bufs=2)
                    for hc in range(8):
                        pt = ps.tile([128, 512], F32, name="psA", tag="psA",
                                     bufs=2)
                        for jp in range(4):
                            nc.tensor.matmul(
                                pt[:, :nw],
                                aw1_t[:, 2 * jp:2 * jp + 2,
                                      hc * 128:(hc + 1) * 128],
                                sT8[:, 2 * jp:2 * jp + 2, :nw],
                                start=(jp == 0), stop=(jp == 3),
                                perf_mode=PM.DoubleRow)
                        nc.scalar.activation(h1a[:, hc, :nw], pt[:, :nw],
                                             AF.Relu, bias=ab1_t[:, hc:hc + 1])
                    # attn l2
                    h2a = tk.tile([128, 8, 256], F8, name="h2a", tag="h2a",
                                  bufs=2)
                    for hc in range(8):
                        pt = ps.tile([128, 512], F32, name="psA", tag="psA",
                                     bufs=2)
                        for jp in range(4):
                            nc.tensor.matmul(
                                pt[:, :nw],
                                aw2_t[:, 2 * jp:2 * jp + 2,
                                      hc * 128:(hc + 1) * 128],
                                h1a[:, 2 * jp:2 * jp + 2, :nw],
                                start=(jp == 0), stop=(jp == 3),
                                perf_mode=PM.DoubleRow)
                        nc.scalar.activation(h2a[:, hc, :nw], pt[:, :nw],
                                             AF.Relu, bias=ab2_t[:, hc:hc + 1],
                                             scale=1.0 / FS)
                    # attn l3 -> exp(logits)
                    pt = ps.tile([1, 512], F32, name="psL", tag="psL", bufs=1)
                    for k in range(8):
                        nc.tensor.matmul(
                            pt[:, :nw],
                            aw3_t[:, k, :],
                            h2a[:, k, :nw],
                            start=(k == 0), stop=(k == 7))
                    expb = tk.tile([1, 256], F32, name="expb", tag="expb",
                                   bufs=2)
                    nc.scalar.activation(expb[:, :nw], pt[:, :nw], AF.Exp,
                                         bias=float(ab3val),
                                         scale=1.0 / (FS * FS))
                    dma(out=expa_d[0:1, n0:n0 + nw], in_=expb[:, :nw])

                    # projections P1/P2/P3 (bf16)
                    for pi in range(3):
                        src = sTb[b] if pi < 2 else eTb[b]
                        for j in range(nw // 128):
                            ch = (n0 + j * 128) // 128
                            for h0 in (0, 512):
                                pt = ps.tile([128, 512], F32, name="psA",
                                             tag="psA", bufs=2)
                                for k in range(8):
                                    nc.tensor.matmul(
                                        pt[:],
                                        src[:, k, j * 128:(j + 1) * 128],
                                        w1_t[pi][:, k, h0:h0 + 512],
                                        start=(k == 0), stop=(k == 7))
                                nc.vector.tensor_copy(
                                    out=Pmats[pi][:, ch, h0:h0 + 512],
                                    in_=pt[:])

            # ---------- span stage (software-pipelined) ----------
            with tc.tile_pool(name="sp", bufs=1) as sp:
                h1big_ref = [None]

                def g_prep(g):
                    KCg = kcs[g]
                    st = {"KCg": KCg, "c0": bases[g] // 128}
                    d_rep = sp.tile([128, 128], F32, name="d_rep",
                                    tag="d_rep", bufs=3)
                    dma(out=d_rep[:],
                        in_=dflat_p[:, g * 128:(g + 1) * 128]
                        .partition_broadcast(128))
                    de_rep = sp.tile([128, 128], F32, name="de_rep",
                                     tag="de_rep", bufs=3)
                    dma(out=de_rep[:],
                        in_=deflat_p[:, g * 128:(g + 1) * 128]
                        .partition_broadcast(128))
                    len_rep = sp.tile([128, 128], F32, name="len_rep",
                                      tag="len_rep", bufs=2)
                    dma(out=len_rep[:],
                        in_=lenflat_p[:, g * 128:(g + 1) * 128]
                        .partition_broadcast(128))
                    e_col = sp.tile([128, KC], F32, name="e_col",
                                    tag="e_col", bufs=2)
                    dma(out=e_col[:, :KCg],
                        in_=AP(tensor=expa_d.tensor, offset=bases[g],
                               ap=[[1, 128], [128, KCg]]))

                    ohT = sp.tile([128, K_WIN], BF, name="ohT", tag="ohT",
                                  bufs=3)
                    oheT = sp.tile([128, K_WIN], BF, name="oheT", tag="oheT",
                                   bufs=3)
                    for kk in range(KCg):
                        nc.vector.tensor_scalar(
                            out=ohT[:, kk * 128:(kk + 1) * 128], in0=d_rep[:],
                            scalar1=iotaC_t[:, kk:kk + 1], scalar2=None,
                            op0=AT.is_equal)
                        nc.vector.tensor_scalar(
                            out=oheT[:, kk * 128:(kk + 1) * 128], in0=de_rep[:],
                            scalar1=iotaC_t[:, kk:kk + 1], scalar2=None,
                            op0=AT.is_equal)
                    ohlT = sp.tile([128, 128], BF, name="ohlT", tag="ohlT",
                                   bufs=3)
                    nc.vector.tensor_scalar(
                        out=ohlT[:], in0=len_rep[:],
                        scalar1=iotaC_t[:, 0:1], scalar2=None, op0=AT.is_equal)

                    # banded exp weights built directly as [tau, n]
                    eb = sp.tile([128, K_WIN], BF, name="eb", tag="eb", bufs=3)
                    x1 = sp.tile([128, 128], BF, name="x1", tag="x1", bufs=2)
                    x2 = sp.tile([128, 128], BF, name="x2", tag="x2", bufs=2)
                    for kk in range(KCg):
                        nc.vector.tensor_scalar(
                            out=x1[:], in0=d_rep[:],
                            scalar1=iotaC_t[:, kk:kk + 1], scalar2=None,
                            op0=AT.is_le)
                        nc.vector.tensor_scalar(
                            out=x2[:], in0=de_rep[:],
                            scalar1=iotaC_t[:, kk:kk + 1],
                            scalar2=e_col[:, kk:kk + 1],
                            op0=AT.is_ge, op1=AT.mult)
                        nc.vector.tensor_tensor(
                            out=eb[:, kk * 128:(kk + 1) * 128], in0=x1[:],
                            in1=x2[:], op=AT.mult)
                    sps = ps.tile([1, 128], F32, name="psS", tag="psS", bufs=2)
                    for kk in range(KCg):
                        nc.tensor.matmul(sps[:], ones128[:],
                                         eb[:, kk * 128:(kk + 1) * 128],
                                         start=(kk == 0), stop=(kk == KCg - 1))
                    srow = sp.tile([1, 128], F32, name="srow", tag="srow",
                                   bufs=3)
                    nc.vector.reciprocal(srow[:], sps[:])
                    dma(out=rr_d[0:1, g * 128:(g + 1) * 128], in_=srow[:])
                    rinv_rep = sp.tile([128, 128], F32, name="rinv_rep",
                                       tag="rinv_rep", bufs=3)
                    dma(out=rinv_rep[:],
                        in_=rr_d[0:1, g * 128:(g + 1) * 128]
                        .partition_broadcast(128))
                    wgT = sp.tile([128, K_WIN], BF, name="wgT", tag="wgT",
                                  bufs=3)
                    for kk in range(KCg):
                        nc.vector.tensor_tensor(
                            out=wgT[:, kk * 128:(kk + 1) * 128],
                            in0=eb[:, kk * 128:(kk + 1) * 128],
                            in1=rinv_rep[:], op=AT.mult)
                    st.update(ohT=ohT, oheT=oheT, ohlT=ohlT, wgT=wgT)
                    return st

                def g_h1(g, st):
                    KCg, c0 = st["KCg"], st["c0"]
                    ohT, oheT, ohlT, wgT = (st["ohT"], st["oheT"],
                                            st["ohlT"], st["wgT"])
                    gcol = (g % 4) * 128
                    if g % 4 == 0:
                        h1big_ref[0] = sp.tile([128, 8, 512], BF, name="h1big",
                                               tag="h1big", bufs=2)
                    h1big = h1big_ref[0]
                    for hc in range(8):
                        hp = ps.tile([128, 128], F32, name="psH", tag="psH",
                                     bufs=2)
                        hs = slice(hc * 128, (hc + 1) * 128)
                        steps = []
                        for kk in range(KCg):
                            ks = slice(kk * 128, (kk + 1) * 128)
                            steps.append((P1[:, c0 + kk, hs], ohT[:, ks]))
                            steps.append((P2[:, c0 + kk, hs], oheT[:, ks]))
                        steps.append((WBpad[:, hc, :], ohlT[:]))
                        for kk in range(KCg):
                            ks = slice(kk * 128, (kk + 1) * 128)
                            steps.append((P3[:, c0 + kk, hs], wgT[:, ks]))
                        for i, (lhsT, rhs) in enumerate(steps):
                            nc.tensor.matmul(hp[:], lhsT, rhs, start=(i == 0),
                                             stop=(i == len(steps) - 1))
                        if hc % 2 == 0:
                            nc.vector.tensor_scalar(
                                out=h1big[:, hc, gcol:gcol + 128], in0=hp[:],
                                scalar1=0.0, scalar2=None, op0=AT.max)
                        else:
                            nc.scalar.activation(
                                h1big[:, hc, gcol:gcol + 128], hp[:], AF.Relu)
                    return h1big

                def l2_block(blk, h1big):
                    b0 = blk * 512
                    h2big = sp.tile([128, 8, 512], BF, name="h2big",
                                    tag="h2big", bufs=2)
                    for h2c in range(8):
                        pt = ps.tile([128, 512], F32, name="psA",
                                     tag="psA", bufs=2)
                        for k in range(8):
                            nc.tensor.matmul(
                                pt[:], w2_t[:, k, h2c * 128:(h2c + 1) * 128],
                                h1big[:, k, :], start=(k == 0), stop=(k == 7))
                        nc.scalar.activation(h2big[:, h2c, :], pt[:], AF.Relu,
                                             bias=b2_t[:, h2c:h2c + 1])
                    pt = ps.tile([1, 512], F32, name="psL", tag="psL", bufs=1)
                    for k in range(8):
                        nc.tensor.matmul(pt[:], w3_t[:, k:k + 1],
                                         h2big[:, k, :], start=(k == 0),
                                         stop=(k == 7))
                    ob = sp.tile([1, 512], F32, name="ob", tag="ob", bufs=2)
                    nc.vector.tensor_scalar(out=ob[:], in0=pt[:],
                                            scalar1=float(b3val),
                                            scalar2=None, op0=AT.add)
                    dma(out=scores_p[:, b0:b0 + 512], in_=ob[:])

                states = [None] * NGROUPS
                states[0] = g_prep(0)
                if NGROUPS > 1:
                    states[1] = g_prep(1)
                for g in range(NGROUPS):
                    h1big = g_h1(g, states[g])
                    states[g] = None
                    if g + 2 < NGROUPS:
                        states[g + 2] = g_prep(g + 2)
                    if g % 4 == 3:
                        l2_block(g // 4, h1big)

    if SPLIT_WAITS:
        _split_waits(nc)
    return nc


def _split_waits(nc, max_waits=1):
    """This walrus build rejects instructions carrying >max_waits sem waits
    ("Too many sync wait commands"). Hoist excess waits onto same-engine
    NoOps placed immediately before the instruction — identical semantics
    (engine queues are in-order)."""
    ctr = [0]
    for f in nc.m.functions:
        for blk in f.blocks:
            out = []
            for ins in blk.instructions:
                si = getattr(ins, "sync_info", None)
                if si is not None and si.on_wait and len(si.on_wait) > max_waits:
                    waits = list(si.on_wait)
                    for w in waits[:-max_waits]:
                        ctr[0] += 1
                        nop = mybir.InstNoOp(
                            name=f"I-wsplit-{ctr[0]}", ins=[], outs=[],
                            sync_info=mybir.SyncInfo(on_wait=[w], on_update=[]),
                        )
                        nop.engine = ins.engine
                        out.append(nop)
                    ins.sync_info = mybir.SyncInfo(
                        on_wait=waits[-max_waits:],
                        on_update=list(si.on_update or []),
                    )
                out.append(ins)
            blk.instructions[:] = out
    return ctr[0]


_CACHE = {}
LAST_EXEC_NS = None
TRACE = False


def _install_ntff_shim():
    try:
        import antenv.axon_hooks  # noqa: F401
        return
    except ImportError:
        pass
    try:
        from trn_agent_boot.trn_boot import _ntff_profile_via_ctypes
        hook = _ntff_profile_via_ctypes("/opt/axon/libaxon_pjrt.so")
    except Exception:
        hook = None
    m1 = types.ModuleType("antenv")
    m2 = types.ModuleType("antenv.axon_hooks")
    m2.get_axon_ntff_profile_hook = lambda: hook
    m2.set_axon_ntff_profile_hook = lambda h: None
    m1.axon_hooks = m2
    sys.modules.setdefault("antenv", m1)
    sys.modules["antenv.axon_hooks"] = m2


def _wlay(w, dt):
    """[K, M] -> [128, 8, M] '(ks p) m' layout."""
    w = np.asarray(w, np.float32)
    K, M = w.shape
    return np.ascontiguousarray(
        w.reshape(K // 128, 128, M).transpose(1, 0, 2)).astype(dt)


def _prepare(inputs):
    inp = {k: np.asarray(v) for k, v in inputs.items()}
    ss = inp["span_starts"].astype(np.int64)
    sl = inp["span_lengths"].astype(np.int64)
    plan = _plan(ss, sl)
    T_cap, bases, kcs = plan["T_cap"], plan["bases"], plan["kcs"]
    KC = max(kcs)
    NB = -(-T_cap // 256)
    b3val = float(np.asarray(inp["score_b3"]).reshape(-1)[0])
    ab3val = float(np.asarray(inp["attn_b3"]).reshape(-1)[0])

    key = (T_cap, tuple(bases), tuple(kcs), b3val, ab3val)
    if key not in _CACHE:
        _CACHE[key] = _build(T_cap, bases, kcs, b3val, ab3val)
    nc = _CACHE[key]

    sw1 = inp["score_w1"].astype(np.float32)
    shared = {
        "aw1": _wlay(inp["attn_w1"] * FS, f8e4).reshape(128, -1),
        "aw2": _wlay(inp["attn_w2"] * FS, f8e4).reshape(128, -1),
        "aw3": _wlay(inp["attn_w3"] * FS, f8e4).reshape(128, 8, 1),
        "ab1m": np.ascontiguousarray(
            inp["attn_b1"].astype(np.float32).reshape(8, 128).T) * FS,
        "ab2m": np.ascontiguousarray(
            inp["attn_b2"].astype(np.float32).reshape(8, 128).T) * FS,
        "w1a": _wlay(sw1[0:1024], bf16).reshape(128, -1),
        "w1b": _wlay(sw1[1024:2048], bf16).reshape(128, -1),
        "w1c": _wlay(sw1[2048:3072], bf16).reshape(128, -1),
        "w1d": np.ascontiguousarray(sw1[3072:3092]).astype(bf16),
        "wtT": np.ascontiguousarray(
            inp["width_table"].astype(np.float32).T).astype(bf16),
        "b1r": inp["score_b1"].astype(np.float32).reshape(1, HID).astype(bf16),
        "w2": _wlay(inp["score_w2"], bf16).reshape(128, -1),
        "b2m": np.ascontiguousarray(
            inp["score_b2"].astype(np.float32).reshape(8, 128).T),
        "w3m": _wlay(inp["score_w3"], bf16).reshape(128, 8),
        "iotaC": np.ascontiguousarray(
            (np.arange(128, dtype=np.float32)[:, None]
             + 128.0 * np.arange(KC, dtype=np.float32)[None, :])),
    }

    states = inp["states"].astype(np.float32)
    embeds = inp["embeds"].astype(np.float32)

    def blocked(xT_pad, dt=bf16):
        # [1024, NB*256] -> [128, NB, 8, 256]
        return np.ascontiguousarray(
            xT_pad.reshape(8, 128, NB, 256).transpose(1, 2, 0, 3)
        ).astype(dt)

    in_maps = []
    for c in range(N_CORES):
        cb = int(plan["core_base"][c])
        stl = np.zeros((D, NB * 256), np.float32)
        eml = np.zeros((D, NB * 256), np.float32)
        hi = min(T, cb + T_cap)
        stl[:, : hi - cb] = states[cb:hi].T
        eml[:, : hi - cb] = embeds[cb:hi].T
        m = dict(shared)
        m["statesTb"] = blocked(stl)
        m["statesT8b"] = blocked(stl, f8e4)
        m["embedsTb"] = blocked(eml)
        d = plan["d"][c].astype(np.float32)
        dl = plan["dl"][c].astype(np.float32)
        ln = plan["ln"][c].astype(np.float32)
        m["dflat"] = d.reshape(1, C)
        m["deflat"] = dl.reshape(1, C)
        m["lenflat"] = ln.reshape(1, C)
        in_maps.append(m)

    return nc, in_maps, plan


def kernel(**inputs):
    global LAST_EXEC_NS
    from concourse.bass_utils import run_bass_kernel_spmd

    nc, in_maps, plan = _prepare(inputs)
    _install_ntff_shim()
    res = run_bass_kernel_spmd(nc, in_maps, list(range(N_CORES)), trace=TRACE)
    LAST_EXEC_NS = res.exec_time_ns

    out = np.empty(NSPAN, np.float32)
    for c in range(N_CORES):
        out[plan["order"][c * C: (c + 1) * C]] = np.asarray(
            res.results[c]["scores"]).reshape(-1)
    return out.reshape(NSPAN, 1)
